# revision 1
# baseline (speedup 1.0000x reference)
"""Trainium2 Bass kernel for nn_MultiHeadAttention_65910568125151.

B=4, S=1024, D=1024, H=16 heads (dk=64). 8 NeuronCores, sharded
batch x head-half: core c handles batch c//2 and heads (c%2)*8..+8.

Per-core program (all matmuls fp16, fp32 PSUM accumulate):
  phase 1: qT = (Wq.T)loc.T @ Qx[b].T   [512, 1024]   (transposed layout)
           kT likewise; v in natural layout [1024, 512]
  phase 2: per head h: scoresT[k,q] = kT_h.T @ qT_h (K=64)
           x = scoresT + combT (comb = host-folded -8*lam*prob - 480*maskT0)
           e = exp(0.125 * x)        (ACT, scale folds the 1/sqrt(dk)/8)
           attn_un/rowsum via one matmul with ones-augmented v  [65, 1024]
           normalize by DRAM-roundtrip broadcast of 1/rowsum
  phase 3: out_partial = attnT.T @ (Wo.T)loc   [1024, 1024]
Host gathers: out[b] = partial[2b] + partial[2b+1] + bo.

All matmul operands are fp16 (PSUM accumulation stays fp32): the
fused 4-byte-dtype matmul lowers to an S3_LW instruction with room for
only one sync wait, which psum-slot-reuse group leaders exceed; fp16
uses the explicit LDWEIGHTS+MATMUL pair where waits distribute fine.
Host pre-converts the sharded inputs to fp16 (also halving DMA bytes).
"""

import numpy as np

_B, _S, _D = 4, 1024, 1024
_P = 128
_DL = 512          # local hidden (8 heads x 64)
_HL = 8            # local heads
_DK = 64
_KC = _D // _P     # 8 contraction chunks, projections
_MQ = _DL // _P    # 4 m-tiles for qT/kT
_MT = _S // _P     # 8 token tiles
_KO = _DL // _P    # 4 contraction chunks, out-proj
_NH = (0, 512)     # free-dim halves


def _build_program(with_bias: bool):
    from collections import deque

    import concourse.mybir as mybir
    import concourse.tile as tile
    from concourse import bacc
    from concourse.alu_op_type import AluOpType

    f32 = mybir.dt.float32
    f16 = mybir.dt.float16
    Copy = mybir.ActivationFunctionType.Copy
    Exp = mybir.ActivationFunctionType.Exp
    S, DL, P, HL, MT, MQ, KC, KO = _S, _DL, _P, _HL, _MT, _MQ, _KC, _KO

    nc = bacc.Bacc()

    xq_d = nc.dram_tensor("xq", [_D, S], f16, kind="ExternalInput")
    xk_d = nc.dram_tensor("xk", [_D, S], f16, kind="ExternalInput")
    xv_d = nc.dram_tensor("xv", [_D, S], f16, kind="ExternalInput")
    wq_d = nc.dram_tensor("wq", [_D, DL], f16, kind="ExternalInput")
    wk_d = nc.dram_tensor("wk", [_D, DL], f16, kind="ExternalInput")
    wv_d = nc.dram_tensor("wv", [_D, DL], f16, kind="ExternalInput")
    wo_d = nc.dram_tensor("wo", [DL, _D], f16, kind="ExternalInput")
    cb_d = nc.dram_tensor("comb", [S, S], f32, kind="ExternalInput")
    if with_bias:
        bq_d = nc.dram_tensor("bq", [1, DL], f16, kind="ExternalInput")
        bk_d = nc.dram_tensor("bk", [1, DL], f16, kind="ExternalInput")
        bv_d = nc.dram_tensor("bv", [1, DL], f16, kind="ExternalInput")
        on_d = nc.dram_tensor("ones_row", [1, DL], f16, kind="ExternalInput")
    out_d = nc.dram_tensor("out", [S, _D], f32, kind="ExternalOutput")

    with tile.TileContext(nc) as tc:
        with (
            tc.tile_pool(name="pw", bufs=2) as pw,
            tc.tile_pool(name="px", bufs=16) as px,
            tc.tile_pool(name="pqk", bufs=1) as pqk,
            tc.tile_pool(name="psm", bufs=6) as psm,
            tc.tile_pool(name="psmall", bufs=2) as psmall,
            tc.tile_pool(name="pdr", bufs=2, space="DRAM") as pdr,
        ):
            # ---- persistent sbuf tiles (fp32r: matmul operands) ----
            qT_t = pqk.tile([P, MQ, S], f16, tag="qT")
            kT_t = pqk.tile([P, MQ, S], f16, tag="kT")
            v_t = pqk.tile([P, MT, HL, _DK + 1], f16, tag="v")
            attnT_t = pqk.tile([P, KO, S], f16, tag="attnT")
            # ones column for the fused rowsum row of the attn matmul
            nc.vector.memset(v_t[:, :, :, _DK:_DK + 1], 1.0)

            if with_bias:
                ones_t = psmall.tile([1, DL], f16, tag="ones", bufs=1)
                nc.sync.dma_start(ones_t[:], on_d[:])
                bias_ts = {}
                for nm, d in (("bq", bq_d), ("bk", bk_d), ("bv", bv_d)):
                    bt = psmall.tile([1, DL], f16, tag=nm, bufs=1)
                    nc.sync.dma_start(bt[:], d[:])
                    bias_ts[nm] = bt

            # ---- weights ----
            def load_w(d, n_chunks, ncols, ppool):
                t = pw.tile([P, n_chunks, ncols], f16, tag="w")
                nc.sync.dma_start(
                    t[:], d[:].rearrange("(c p) n -> p c n", p=P))
                return t

            # ---- phase 1: projections ----
            with tc.tile_pool(name="pp1", bufs=2, space="PSUM") as pp1:
                # q and k projections (transposed layout)
                for which, x_d, w_d_, bias_nm, dst in (
                    ("q", xq_d, wq_d, "bq", qT_t),
                    ("k", xk_d, wk_d, "bk", kT_t),
                ):
                    w_t = load_w(w_d_, KC, DL, pp1)
                    x_ts = []
                    for kc in range(KC):
                        xt = px.tile([P, S], f16, tag="x")
                        nc.sync.dma_start(xt[:], x_d[kc * P:(kc + 1) * P, :])
                        x_ts.append(xt)
                    if which == "k":
                        # dummy LDW absorbs the fresh-x DMA wait so the
                        # group-leader matmul stays within its 2 wait slots
                        nc.tensor.ldweights(x_ts[0][:, 0:P])
                    for m in range(MQ):
                        pq = pp1.tile([P, S], f32, tag="pq")
                        for kc in range(KC):
                            lhsT = w_t[:, kc, m * P:(m + 1) * P]
                            for o in _NH:
                                nc.tensor.matmul(
                                    pq[:, o:o + 512], lhsT,
                                    x_ts[kc][:, o:o + 512],
                                    start=(kc == 0),
                                    stop=(kc == KC - 1 and not with_bias),
                                )
                        if with_bias:
                            bt = bias_ts[bias_nm]
                            for o in _NH:
                                nc.tensor.matmul(
                                    pq[:, o:o + 512],
                                    bt[0:1, m * P:(m + 1) * P],
                                    ones_t[0:1, 0:512],
                                    start=False, stop=True,
                                )
                        nc.scalar.activation(dst[:, m, :], pq[:], Copy)

                # v projection (natural layout) + comb prefetch
                wv_t = load_w(wv_d, KC, DL, pp1)
                xv_ts = []
                for kc in range(KC):
                    xt = px.tile([P, S], f16, tag="x")
                    nc.sync.dma_start(xt[:], xv_d[kc * P:(kc + 1) * P, :])
                    xv_ts.append(xt)
                cb_ts = []
                for kt in range(MT):
                    ct = px.tile([P, S], f32, tag="x")
                    nc.sync.dma_start(ct[:], cb_d[kt * P:(kt + 1) * P, :])
                    cb_ts.append(ct)
                for mt in range(MT):
                    pv = pp1.tile([P, DL], f32, tag="pv")
                    for kc in range(KC):
                        nc.tensor.matmul(
                            pv[:],
                            xv_ts[kc][:, mt * P:(mt + 1) * P],
                            wv_t[:, kc, :],
                            start=(kc == 0),
                            stop=(kc == KC - 1 and not with_bias),
                        )
                    if with_bias:
                        nc.tensor.matmul(
                            pv[:], ones_t[0:1, 0:P], bias_ts["bv"][0:1, :],
                            start=False, stop=True,
                        )
                    nc.scalar.activation(
                        v_t[:, mt, :, 0:_DK],
                        pv[:].rearrange("p (h d) -> p h d", h=HL),
                        Copy,
                    )

            # ---- phase 2: attention per head ----
            with tc.tile_pool(name="pp2", bufs=2, space="PSUM") as pp2:
                pending = deque()

                def norm_tail(h, pa):
                    mq, base = h // 2, (h % 2) * 64
                    rc = psmall.tile([1, S], f32, tag="rc")
                    nc.vector.reciprocal(rc[0:1, :], pa[64:65, :])
                    rb_d = pdr.tile([1, S], f32, tag="rbd")
                    nc.sync.dma_start(rb_d[:], rc[0:1, :])
                    rb = psm.tile([64, S], f32, tag="rb", bufs=2)
                    nc.sync.dma_start(rb[:], rb_d[0:1, :].partition_broadcast(64))
                    nc.vector.tensor_tensor(
                        out=attnT_t[base:base + 64, mq, :],
                        in0=pa[0:64, :], in1=rb[:], op=AluOpType.mult,
                    )

                def flush_one():
                    h, kt, e, pa = pending.popleft()
                    vh = v_t[:, kt, h, :]
                    for o in _NH:
                        nc.tensor.matmul(
                            pa[:, o:o + 512], vh, e[:, o:o + 512],
                            start=(kt == 0), stop=(kt == MT - 1),
                        )
                    if kt == MT - 1:
                        norm_tail(h, pa)

                for h in range(HL):
                    mq, base = h // 2, (h % 2) * 64
                    kTh = kT_t[base:base + 64, mq, :]
                    qTh = qT_t[base:base + 64, mq, :]
                    pa = pp2.tile([65, S], f32, tag="pa")
                    for kt in range(MT):
                        ps_ = pp2.tile([P, S], f32, tag="ps")
                        lhsT = kTh[:, kt * P:(kt + 1) * P]
                        for o in _NH:
                            nc.tensor.matmul(
                                ps_[:, o:o + 512], lhsT, qTh[:, o:o + 512],
                                start=True, stop=True,
                            )
                        x_sb = psm.tile([P, S], f32, tag="sm")
                        nc.vector.tensor_tensor(
                            out=x_sb[:], in0=ps_[:], in1=cb_ts[kt][:],
                            op=AluOpType.add,
                        )
                        e_sb = psm.tile([P, S], f16, tag="sme", bufs=4)
                        nc.scalar.activation(e_sb[:], x_sb[:], Exp, scale=0.125)
                        pending.append((h, kt, e_sb, pa))
                        if len(pending) > 2:
                            flush_one()
                while pending:
                    flush_one()

            # ---- phase 3: output projection ----
            with tc.tile_pool(name="pp3", bufs=2, space="PSUM") as pp3:
                wo_t = load_w(wo_d, KO, _D, pp3)
                nc.tensor.ldweights(wo_t[:, 0, 0:P])
                for mt in range(MT):
                    po = pp3.tile([P, _D], f32, tag="po")
                    for ko in range(KO):
                        lhsT = attnT_t[:, ko, mt * P:(mt + 1) * P]
                        for o in _NH:
                            nc.tensor.matmul(
                                po[:, o:o + 512], lhsT, wo_t[:, ko, o:o + 512],
                                start=(ko == 0), stop=(ko == KO - 1),
                            )
                    o_sb = psm.tile([P, _D], f32, tag="sm")
                    nc.scalar.activation(o_sb[:], po[:], Copy)
                    nc.sync.dma_start(out_d[mt * P:(mt + 1) * P, :], o_sb[:])

    nc.compile()
    return nc


_PROG_CACHE = {}


def _get_program(with_bias: bool):
    if with_bias not in _PROG_CACHE:
        _PROG_CACHE[with_bias] = _build_program(with_bias)
    return _PROG_CACHE[with_bias]


def _prepare_in_maps(Qx, Kx, Vx, prob_phn, mask, lambda_val,
                     Wq, bq, Wk, bk, Wv, bv, Wo, bo):
    f32 = np.float32
    Qx = np.asarray(Qx, f32)
    Kx = np.asarray(Kx, f32)
    Vx = np.asarray(Vx, f32)
    prob = np.asarray(prob_phn, f32)
    mask_np = np.asarray(mask)
    lam = float(np.asarray(lambda_val))

    f16 = np.float16
    QxT = np.ascontiguousarray(Qx.transpose(0, 2, 1)).astype(f16)
    KxT = np.ascontiguousarray(Kx.transpose(0, 2, 1)).astype(f16)
    VxT = np.ascontiguousarray(Vx.transpose(0, 2, 1)).astype(f16)
    WqT = np.ascontiguousarray(np.asarray(Wq, f32).T).astype(f16)
    WkT = np.ascontiguousarray(np.asarray(Wk, f32).T).astype(f16)
    WvT = np.ascontiguousarray(np.asarray(Wv, f32).T).astype(f16)
    WoT = np.ascontiguousarray(np.asarray(Wo, f32).T).astype(f16)

    # comb[k, q] = 8 * (-lam * prob[k, q] - 60 * (mask[q, k] == 0))
    comb = (f32(-8.0 * lam) * prob)
    comb = comb + np.where(mask_np.transpose(0, 2, 1) == 0, f32(-480.0), f32(0.0))
    comb = np.ascontiguousarray(comb, dtype=f32)

    bq = np.asarray(bq, f32)
    bk = np.asarray(bk, f32)
    bv = np.asarray(bv, f32)
    with_bias = bool(bq.any() or bk.any() or bv.any())

    in_maps = []
    for c in range(8):
        b, hh = divmod(c, 2)
        sl = slice(hh * _DL, (hh + 1) * _DL)
        m = {
            "xq": QxT[b], "xk": KxT[b], "xv": VxT[b], "comb": comb[b],
            "wq": np.ascontiguousarray(WqT[:, sl]),
            "wk": np.ascontiguousarray(WkT[:, sl]),
            "wv": np.ascontiguousarray(WvT[:, sl]),
            "wo": np.ascontiguousarray(WoT[sl, :]),
        }
        if with_bias:
            m["bq"] = np.ascontiguousarray(bq[sl]).reshape(1, _DL).astype(f16)
            m["bk"] = np.ascontiguousarray(bk[sl]).reshape(1, _DL).astype(f16)
            m["bv"] = np.ascontiguousarray(bv[sl]).reshape(1, _DL).astype(f16)
            m["ones_row"] = np.ones((1, _DL), f16)
        in_maps.append(m)
    return in_maps, with_bias, mask_np, np.asarray(bo, f32)


def _run(trace=False, tmpdir=None, **inputs):
    from concourse.bass_utils import run_bass_kernel_spmd

    in_maps, with_bias, mask_np, bo = _prepare_in_maps(**inputs)
    nc = _get_program(with_bias)
    br = run_bass_kernel_spmd(nc, in_maps, list(range(8)), trace=trace,
                              tmpdir=tmpdir)
    out = np.empty((_B, _S, _D), np.float32)
    for b in range(_B):
        out[b] = br.results[2 * b]["out"] + br.results[2 * b + 1]["out"]
    out += bo
    return (out, mask_np), br


def kernel(**inputs):
    (out, mask_np), _ = _run(trace=False, **inputs)
    return out, mask_np



# revision 25
# speedup vs baseline: 1.2138x; 1.2138x over previous
"""Trainium2 Bass kernel for nn_MultiHeadAttention_65910568125151.

B=4, S=1024, D=1024, H=16 heads (dk=64). 8 NeuronCores, sharded
batch x head-half: core c handles batch c//2 and heads (c%2)*8..+8.

Per-core program (all matmuls fp16, fp32 PSUM accumulate):
  phase 1: qT = (Wq.T)loc.T @ Qx[b].T   [512, 1024]   (transposed layout)
           kT likewise; v in natural layout, widened with 64 ones
           columns per head so the attn matmul emits the softmax
           rowsum replicated over psum partitions 64:128 for free.
  phase 2 (per head-pair m = chunk m of qT/kT):
           scoresT[k,q] = kT_h.T @ qT_h (K=64); the two heads of the
           pair sit in array row-groups 0-63 / 64-127 (tile_position
           auto-derived) and run concurrently.
           e_raw = exp(scoresT/8)  (ACT, direct from PSUM, scale=1/8)
           e = e_raw * ec          (DVE f16 2x mode; ec precomputed on
                                    host = exp(-lam*prob)*(maskT!=0))
           pa[0:64]  = unnormalized attn, pa[64:128] = rowsum (ones cols)
           rc = reciprocal_approx_fast(pa[64:128])   (DVE, [64,S])
           attnT = pa[0:64] * rc   (DVE)  -- no broadcast DMA needed
  phase 3: out = attnT.T @ (Wo.T)loc, f16 partials to DRAM
Host gathers: out[b] = partial[2b] + partial[2b+1] + bo (fp32).

Emission order interleaves projections with attention pairs so the
scalar engine (exp is the phase-2 bottleneck) starts early and the PE
stays warm: wq/wk + chunk0 proj -> pair0 scores/exp -> v proj ->
pair0 attn -> chunk1 proj -> pair1 -> ...
"""

import numpy as np

_B, _S, _D = 4, 1024, 1024
_P = 128
_DL = 512          # local hidden (8 heads x 64)
_HL = 8            # local heads
_DK = 64
_KC = _D // _P     # 8 contraction chunks, projections
_MQ = _DL // _P    # 4 m-tiles (head pairs) for qT/kT
_MT = _S // _P     # 8 token tiles
_KO = _DL // _P    # 4 contraction chunks, out-proj
_NH = (0, 512)     # free-dim halves


def _build_program(with_bias: bool):
    import concourse.mybir as mybir
    import concourse.tile as tile
    from concourse import bacc
    from concourse.alu_op_type import AluOpType

    f32 = mybir.dt.float32
    f16 = mybir.dt.float16
    Copy = mybir.ActivationFunctionType.Copy
    Exp = mybir.ActivationFunctionType.Exp
    S, DL, P, HL, MT, MQ, KC, KO = _S, _DL, _P, _HL, _MT, _MQ, _KC, _KO

    nc = bacc.Bacc()

    xq_d = nc.dram_tensor("xq", [_D, S], f16, kind="ExternalInput")
    xk_d = nc.dram_tensor("xk", [_D, S], f16, kind="ExternalInput")
    xv_d = nc.dram_tensor("xv", [_D, S], f16, kind="ExternalInput")
    wq_d = nc.dram_tensor("wq", [_D, DL], f16, kind="ExternalInput")
    wk_d = nc.dram_tensor("wk", [_D, DL], f16, kind="ExternalInput")
    wv_d = nc.dram_tensor("wv", [_D, DL], f16, kind="ExternalInput")
    wo_d = nc.dram_tensor("wo", [DL, _D], f16, kind="ExternalInput")
    ec_d = nc.dram_tensor("ec", [S, S], f16, kind="ExternalInput")
    if with_bias:
        bq_d = nc.dram_tensor("bq", [1, DL], f16, kind="ExternalInput")
        bk_d = nc.dram_tensor("bk", [1, DL], f16, kind="ExternalInput")
        bv_d = nc.dram_tensor("bv", [1, DL], f16, kind="ExternalInput")
        on_d = nc.dram_tensor("ones_row", [1, DL], f16, kind="ExternalInput")
    out_d = nc.dram_tensor("out", [S, _D], f16, kind="ExternalOutput")

    with tile.TileContext(nc) as tc:
        with (
            tc.tile_pool(name="pw", bufs=4) as pw,
            tc.tile_pool(name="px", bufs=24) as px,
            tc.tile_pool(name="pqk", bufs=1) as pqk,
            tc.tile_pool(name="psm", bufs=2) as psm,
            tc.tile_pool(name="psmall", bufs=2) as psmall,
        ):
            # ---- persistent sbuf tiles ----
            qT_t = pqk.tile([P, MQ, S], f16, tag="qT")
            kT_t = pqk.tile([P, MQ, S], f16, tag="kT")
            # v2: per (token-tile, head): [v (64 cols) | ones (64 cols)]
            v2_t = pqk.tile([P, MT, HL, P], f16, tag="v2")
            attnT_t = pqk.tile([P, KO, S], f16, tag="attnT")
            nc.vector.memset(v2_t[:, :, :, _DK:], 1.0)

            if with_bias:
                ones_t = psmall.tile([1, DL], f16, tag="ones", bufs=1)
                nc.sync.dma_start(ones_t[:], on_d[:])
                bias_ts = {}
                for nm, d in (("bq", bq_d), ("bk", bk_d), ("bv", bv_d)):
                    bt = psmall.tile([1, DL], f16, tag=nm, bufs=1)
                    nc.sync.dma_start(bt[:], d[:])
                    bias_ts[nm] = bt

            def load_w(d, n_chunks, ncols):
                t = pw.tile([P, n_chunks, ncols], f16, tag="w")
                nc.sync.dma_start(
                    t[:], d[:].rearrange("(c p) n -> p c n", p=P))
                return t

            # input DMAs (issued up front; scheduler overlaps)
            wq_t = load_w(wq_d, KC, DL)
            wk_t = load_w(wk_d, KC, DL)
            xq_ts, xk_ts = [], []
            for x_d, lst in ((xq_d, xq_ts), (xk_d, xk_ts)):
                for kc in range(KC):
                    xt = px.tile([P, S], f16, tag="x")
                    nc.sync.dma_start(xt[:], x_d[kc * P:(kc + 1) * P, :])
                    lst.append(xt)
            wv_t = load_w(wv_d, KC, DL)
            xv_ts = []
            for kc in range(KC):
                xt = px.tile([P, S], f16, tag="x")
                nc.sync.dma_start(xt[:], xv_d[kc * P:(kc + 1) * P, :])
                xv_ts.append(xt)
            ec_ts = []
            for kt in range(MT):
                ct = px.tile([P, S], f16, tag="ec", bufs=8)
                nc.sync.dma_start(ct[:], ec_d[kt * P:(kt + 1) * P, :])
                ec_ts.append(ct)

            with tc.tile_pool(name="pp", bufs=2, space="PSUM") as pp:

                # PSUM budget (8 banks of [128, 2KB]):
                #   tag "ps"  [128,1024] f32 x2 bufs = 4 banks
                #     (shared by q/k-proj psum, scores psum, out-proj psum)
                #   tags "pa0"/"pa1" [128,1024] f32 x1 buf = 4 banks
                #     (v-proj psum rotates through them before the pairs
                #      claim them for attn accumulation)
                def proj_qk(m, w_t, x_ts, bias_nm, dst):
                    """project chunk m (one head pair) of q or k."""
                    pq = pp.tile([P, S], f32, tag="ps", bufs=2)
                    for kc in range(KC):
                        lhsT = w_t[:, kc, m * P:(m + 1) * P]
                        for o in _NH:
                            nc.tensor.matmul(
                                pq[:, o:o + 512], lhsT,
                                x_ts[kc][:, o:o + 512],
                                start=(kc == 0),
                                stop=(kc == KC - 1 and not with_bias),
                            )
                    if with_bias:
                        bt = bias_ts[bias_nm]
                        for o in _NH:
                            nc.tensor.matmul(
                                pq[:, o:o + 512],
                                bt[0:1, m * P:(m + 1) * P],
                                ones_t[0:1, 0:512],
                                start=False, stop=True,
                            )
                    nc.scalar.activation(dst[:, m, :], pq[:], Copy)

                def proj_v_chunk(mt):
                    pvf = pp.tile([P, S], f32, tag=f"pa{mt % 2}",
                                  bufs=1, name=f"pv{mt}")
                    pv = pvf[:, 0:DL]
                    for kc in range(KC):
                        nc.tensor.matmul(
                            pv,
                            xv_ts[kc][:, mt * P:(mt + 1) * P],
                            wv_t[:, kc, :],
                            start=(kc == 0),
                            stop=(kc == KC - 1 and not with_bias),
                        )
                    if with_bias:
                        nc.tensor.matmul(
                            pv, ones_t[0:1, 0:P],
                            bias_ts["bv"][0:1, :],
                            start=False, stop=True,
                        )
                    nc.vector.tensor_copy(
                        v2_t[:, mt, :, 0:_DK],
                        pv.rearrange("p (h d) -> p h d", h=HL),
                    )

                # ---- phase 2: attention, one head-pair per qT/kT chunk ----
                def pair_scores(m, kt):
                    """scores+exp+mult for pair m, token tile kt.
                    Returns (e_even, e_odd) f16 tiles."""
                    es = []
                    ps_pair = []
                    for base in (0, 64):
                        ps_ = pp.tile([P, S], f32, tag="ps", bufs=2)
                        ps_pair.append(ps_)
                    # interleave even/odd MMs so the two row-groups of
                    # the PE array run concurrently
                    for o in _NH:
                        for i, base in enumerate((0, 64)):
                            lhsT = kT_t[base:base + 64, m, kt * P:(kt + 1) * P]
                            rhs = qT_t[base:base + 64, m, o:o + 512]
                            nc.tensor.matmul(
                                ps_pair[i][:, o:o + 512], lhsT, rhs,
                                start=True, stop=True,
                            )
                    for i, base in enumerate((0, 64)):
                        er = psm.tile([P, S], f16, tag="eraw", bufs=2)
                        nc.scalar.activation(er[:], ps_pair[i][:], Exp,
                                             scale=0.125)
                        e_sb = psm.tile([P, S], f16, tag="e", bufs=16)
                        nc.vector.tensor_tensor(
                            out=e_sb[:], in0=er[:], in1=ec_ts[kt][:],
                            op=AluOpType.mult,
                        )
                        es.append(e_sb)
                    return es

                def pair_attn(m, kt, pa_pair, es):
                    for i in range(2):
                        vh = v2_t[:, kt, 2 * m + i, :]
                        for o in _NH:
                            nc.tensor.matmul(
                                pa_pair[i][:, o:o + 512], vh,
                                es[i][:, o:o + 512],
                                start=(kt == 0), stop=(kt == MT - 1),
                            )

                def pair_norm(m, pa_pair):
                    for i, base in enumerate((0, 64)):
                        # custom-DVE recip mishandles base_partition=64
                        # inputs; stage rowsums to a base-0 SBUF tile first
                        rs = psm.tile([64, S], f32, tag="rs", bufs=2)
                        nc.vector.tensor_copy(rs[:], pa_pair[i][64:128, :])
                        rc = psm.tile([64, S], f32, tag="rc", bufs=2)
                        nc.vector.reciprocal_approx_fast(rc[:], rs[:])
                        nc.vector.tensor_tensor(
                            out=attnT_t[base:base + 64, m, :],
                            in0=pa_pair[i][0:64, :], in1=rc[:],
                            op=AluOpType.mult,
                        )

                # pair 0: interleave v-projection chunks with the score
                # tiles (both PE work) so ACT starts exp'ing early; the
                # attn flushes come after v2 is complete. pa tiles are
                # allocated AFTER the v-proj psum (they rotate the same
                # tags).
                proj_qk(0, wq_t, xq_ts, "bq", qT_t)
                proj_qk(0, wk_t, xk_ts, "bk", kT_t)
                es0 = []
                for kt in range(MT):
                    es0.append(pair_scores(0, kt))
                    proj_v_chunk(kt)
                pa0 = [pp.tile([P, S], f32, tag=f"pa{i}", bufs=1,
                                name=f"pa0_{i}")
                       for i in range(2)]
                for kt in range(MT):
                    pair_attn(0, kt, pa0, es0[kt])
                pair_norm(0, pa0)

                # pairs 1..3: pipelined scores/attn with lag 2
                for m in range(1, MQ):
                    proj_qk(m, wq_t, xq_ts, "bq", qT_t)
                    proj_qk(m, wk_t, xk_ts, "bk", kT_t)
                    pa = [pp.tile([P, S], f32, tag=f"pa{i}", bufs=1,
                                  name=f"pa{m}_{i}")
                          for i in range(2)]
                    pend = []
                    for kt in range(MT):
                        es = pair_scores(m, kt)
                        pend.append((kt, es))
                        if len(pend) > 2:
                            kt_, es_ = pend.pop(0)
                            pair_attn(m, kt_, pa, es_)
                    for kt_, es_ in pend:
                        pair_attn(m, kt_, pa, es_)
                    pair_norm(m, pa)

                # ---- phase 3: output projection ----
                wo_t = load_w(wo_d, KO, _D)
                nc.tensor.ldweights(wo_t[:, 0, 0:P])
                for mt in range(MT):
                    po = pp.tile([P, _D], f32, tag="ps", bufs=2)
                    for ko in range(KO):
                        lhsT = attnT_t[:, ko, mt * P:(mt + 1) * P]
                        for o in _NH:
                            nc.tensor.matmul(
                                po[:, o:o + 512], lhsT,
                                wo_t[:, ko, o:o + 512],
                                start=(ko == 0), stop=(ko == KO - 1),
                            )
                    o_sb = psm.tile([P, _D], f16, tag="osb", bufs=2)
                    nc.scalar.activation(o_sb[:], po[:], Copy)
                    nc.sync.dma_start(out_d[mt * P:(mt + 1) * P, :], o_sb[:])

    nc.compile()
    return nc


_PROG_CACHE = {}


def _get_program(with_bias: bool):
    if with_bias not in _PROG_CACHE:
        _PROG_CACHE[with_bias] = _build_program(with_bias)
    return _PROG_CACHE[with_bias]


def _prepare_in_maps(Qx, Kx, Vx, prob_phn, mask, lambda_val,
                     Wq, bq, Wk, bk, Wv, bv, Wo, bo):
    f32 = np.float32
    Qx = np.asarray(Qx, f32)
    Kx = np.asarray(Kx, f32)
    Vx = np.asarray(Vx, f32)
    prob = np.asarray(prob_phn, f32)
    mask_np = np.asarray(mask)
    lam = float(np.asarray(lambda_val))

    f16 = np.float16
    QxT = np.ascontiguousarray(Qx.transpose(0, 2, 1)).astype(f16)
    KxT = np.ascontiguousarray(Kx.transpose(0, 2, 1)).astype(f16)
    VxT = np.ascontiguousarray(Vx.transpose(0, 2, 1)).astype(f16)
    WqT = np.ascontiguousarray(np.asarray(Wq, f32).T).astype(f16)
    WkT = np.ascontiguousarray(np.asarray(Wk, f32).T).astype(f16)
    WvT = np.ascontiguousarray(np.asarray(Wv, f32).T).astype(f16)
    WoT = np.ascontiguousarray(np.asarray(Wo, f32).T).astype(f16)

    # ec[k, q] = exp(-lam * prob[k, q]) * (mask[q, k] != 0)
    # (masked entries become exact 0; e' = exp(scoresT/8) * ec)
    ec = np.exp(-lam * prob) * (mask_np.transpose(0, 2, 1) != 0)
    ec = np.ascontiguousarray(ec, dtype=f16)

    bq = np.asarray(bq, f32)
    bk = np.asarray(bk, f32)
    bv = np.asarray(bv, f32)
    with_bias = bool(bq.any() or bk.any() or bv.any())

    in_maps = []
    for c in range(8):
        b, hh = divmod(c, 2)
        sl = slice(hh * _DL, (hh + 1) * _DL)
        m = {
            "xq": QxT[b], "xk": KxT[b], "xv": VxT[b], "ec": ec[b],
            "wq": np.ascontiguousarray(WqT[:, sl]),
            "wk": np.ascontiguousarray(WkT[:, sl]),
            "wv": np.ascontiguousarray(WvT[:, sl]),
            "wo": np.ascontiguousarray(WoT[sl, :]),
        }
        if with_bias:
            m["bq"] = np.ascontiguousarray(bq[sl]).reshape(1, _DL).astype(f16)
            m["bk"] = np.ascontiguousarray(bk[sl]).reshape(1, _DL).astype(f16)
            m["bv"] = np.ascontiguousarray(bv[sl]).reshape(1, _DL).astype(f16)
            m["ones_row"] = np.ones((1, _DL), f16)
        in_maps.append(m)
    return in_maps, with_bias, mask_np, np.asarray(bo, f32)


def _run(trace=False, tmpdir=None, **inputs):
    from concourse.bass_utils import run_bass_kernel_spmd

    in_maps, with_bias, mask_np, bo = _prepare_in_maps(**inputs)
    nc = _get_program(with_bias)
    br = run_bass_kernel_spmd(nc, in_maps, list(range(8)), trace=trace,
                              tmpdir=tmpdir)
    out = np.empty((_B, _S, _D), np.float32)
    for b in range(_B):
        out[b] = br.results[2 * b]["out"].astype(np.float32)
        out[b] += br.results[2 * b + 1]["out"].astype(np.float32)
    out += bo
    return (out, mask_np), br


def kernel(**inputs):
    (out, mask_np), _ = _run(trace=False, **inputs)
    return out, mask_np
